# revision 13
# baseline (speedup 1.0000x reference)
"""DeltaTokenShift Trainium2 kernel (Bass/Tile, 8 NeuronCores via axon).

Computation (per batch b):
    erase = sigmoid(x @ We + be) ; write = sigmoid(x @ Ww + bw)
    s_t = s_{t-1} * (1 - erase_t) + write_t * x_t   (scan over L, per channel)
    out[:, t, :] = s_t

Sharding: 8 cores = 4 batches x 2 halves of the 1024-channel dim. Each core
gets x[b] pre-transposed on the host to xT (the gate matmul contracts over
all 1024 input channels), its 512-column weight slices, bias/state slices,
and computes out[b][:, half] in channel-major layout [512, 4096]; the host
transposes back at gather. For upper-half cores, xT rows and weight rows are
rotated by 512 so the core's own gate channels always occupy xT k-tiles 0..3
(a consistent permutation of the contraction dim leaves the matmul result
unchanged).

Host-side packing puts the contraction k-tile index in the middle dim
([P, KT, *]) so a whole chunk of xT (all 8 k-tiles) moves with a single
dma_start, and each weight matrix with one more.

With xT supplied by the host, the PE does nothing but the gate matmuls
(f32r, 1 cyc/row): rhs tiles come straight from DMA, the b-term w*x reads
the same f32 tiles on GpSimd, the DVE scan output DMAs directly to DRAM in
[e, l] layout. No on-chip transposes, no PSUM->SBUF staging copies.

Per-core pipeline over 512-token L-chunks:
  DMA xT chunk [128, 8, 512] (sync queue, double buffered, 1 issue/chunk)
  gate matmuls in [e, l] layout accumulate in PSUM (f32r weights resident)
  ACT sigmoid straight from PSUM (erase uses scale=-1, bias=-be => 1-sigmoid)
  GpSimd b = write * xT[m] ; DVE tensor_tensor_scan(a, b) chained across
  chunks via initial=prev[:, -1:] ; DMA s tile -> out[e-block, l-chunk].
"""

import sys

sys.path.insert(0, "/opt/trn_rl_repo")

import numpy as np
import concourse.bacc as bacc
import concourse.mybir as mybir
from concourse.tile import TileContext
from concourse.bass_utils import run_bass_kernel_spmd

B, L = 4, 4096

F32 = mybir.dt.float32
F32R = mybir.dt.float32r
F8E4 = mybir.dt.float8e4
F8E5 = mybir.dt.float8e5

P = 128
DIN = 1024
ESH = 512
KT = DIN // P  # 8 contraction k-tiles
MT = ESH // P  # 4 output-channel groups per core


def _build_kernel_impl(L=4096, lc=512, chunks=None, ps_bufs=6, wdt=F32R,
                       b_eng="vector", out_queue="sync"):
    if chunks is None:
        chunks = [lc] * (L // lc)
    assert sum(chunks) == L and all(c % P == 0 and c <= lc for c in chunks)

    nc = bacc.Bacc("TRN2", target_bir_lowering=False)

    # Host-packed layouts: element (p, k, j) = xT[k*128 + p, j] etc.
    xt = nc.dram_tensor("xt", [P, KT, L], F32R, kind="ExternalInput")
    we = nc.dram_tensor("we", [P, KT, ESH], wdt, kind="ExternalInput")
    ww = nc.dram_tensor("ww", [P, KT, ESH], wdt, kind="ExternalInput")
    # consts[:, m] = -erase_bias group m ; [:, MT+m] = +write_bias group m ;
    # [:, 2*MT+m] = initial state group m
    consts = nc.dram_tensor("consts", [P, 3 * MT], F32, kind="ExternalInput")
    out = nc.dram_tensor("out", [ESH, L], F32, kind="ExternalOutput")

    with TileContext(nc) as tc:
        with (
            tc.tile_pool(name="const", bufs=1) as constp,
            tc.tile_pool(name="wsb", bufs=1) as wsb,
            tc.tile_pool(name="xt", bufs=2) as xtp,
            tc.tile_pool(name="gate", bufs=3) as gatep,
            tc.tile_pool(name="bmul", bufs=4) as bmulp,
            tc.tile_pool(name="scan", bufs=2) as scanp,
            tc.tile_pool(name="ps_mm", bufs=ps_bufs, space="PSUM") as ps_mm,
        ):
            const_sb = constp.tile([P, 3 * MT], F32, tag="consts")
            bias_sb = const_sb[:, :2 * MT]
            st_sb = const_sb[:, 2 * MT:]

            # Chunk-0 x k-tiles on the sync queue, weights k-tiles on the
            # scalar HWDGE queue: fine-grained interleaved arrival so the
            # first accumulation chains start as soon as their pair lands.
            xts0 = xtp.tile([P, KT, lc], F32R, tag="xt")
            we_sb = wsb.tile([P, KT, ESH], wdt, tag="w0")
            ww_sb = wsb.tile([P, KT, ESH], wdt, tag="w1")
            w_sb = [we_sb, ww_sb]
            for k in range(KT):
                nc.sync.dma_start(xts0[:, k, :chunks[0]],
                                  xt[:, k, :chunks[0]])
                for gi, wt in enumerate((we, ww)):
                    nc.scalar.dma_start(w_sb[gi][:, k, :], wt[:, k, :])
                if k == 0:
                    nc.scalar.dma_start(const_sb[:], consts[:])

            prev_s = [None] * MT
            prev_lc = 0
            l0 = 0

            for c, lcc in enumerate(chunks):
                if c == 0:
                    xts = xts0
                else:
                    xts = xtp.tile([P, KT, lc], F32R, tag="xt")
                    if c == 1:
                        # startup is DMA-bandwidth-bound: per-k granularity
                        # lets the PE chase individual k-tile arrivals
                        for k in range(KT):
                            nc.sync.dma_start(
                                xts[:, k, :lcc], xt[:, k, l0:l0 + lcc])
                    else:
                        nc.sync.dma_start(
                            xts[:, :, :lcc], xt[:, :, l0:l0 + lcc])

                for m in range(MT):
                    pe = ps_mm.tile([P, lc], F32, tag="psmm")
                    for k in range(KT):
                        nc.tensor.matmul(
                            pe[:, :lcc],
                            w_sb[0][:, k, m * P:(m + 1) * P],
                            xts[:, k, :lcc],
                            start=(k == 0), stop=(k == KT - 1),
                        )
                    a_t = gatep.tile([P, lc], F32, tag="a")
                    nc.scalar.activation(
                        a_t[:, :lcc], pe[:, :lcc],
                        mybir.ActivationFunctionType.Sigmoid,
                        bias=bias_sb[:, m:m + 1], scale=-1.0,
                    )

                    pw = ps_mm.tile([P, lc], F32, tag="psmm")
                    for k in range(KT):
                        nc.tensor.matmul(
                            pw[:, :lcc],
                            w_sb[1][:, k, m * P:(m + 1) * P],
                            xts[:, k, :lcc],
                            start=(k == 0), stop=(k == KT - 1),
                        )
                    w_t = gatep.tile([P, lc], F32, tag="w")
                    nc.scalar.activation(
                        w_t[:, :lcc], pw[:, :lcc],
                        mybir.ActivationFunctionType.Sigmoid,
                        bias=bias_sb[:, MT + m:MT + m + 1], scale=1.0,
                    )

                    b_t = bmulp.tile([P, lc], F32, tag="b")
                    getattr(nc, b_eng).tensor_tensor(
                        b_t[:, :lcc], w_t[:, :lcc],
                        xts[:, m, :lcc].bitcast(F32),
                        op=mybir.AluOpType.mult)

                    s_t = scanp.tile([P, lc], F32, tag=f"s{m}")
                    init = st_sb[:, m:m + 1] if c == 0 else \
                        prev_s[m][:, prev_lc - 1:prev_lc]
                    nc.vector.tensor_tensor_scan(
                        s_t[:, :lcc], a_t[:, :lcc], b_t[:, :lcc], init,
                        op0=mybir.AluOpType.mult, op1=mybir.AluOpType.add,
                    )
                    prev_s[m] = s_t

                    # Channel-major store straight from the scan tile; the
                    # host transposes back at gather time.
                    getattr(nc, out_queue).dma_start(
                        out[m * P:(m + 1) * P, l0:l0 + lcc], s_t[:, :lcc])

                prev_lc = lcc
                l0 += lcc

    nc.finalize()
    return nc




def _build_kernel_fp8(L=4096, lc=512, chunks=None, ps_bufs=6,
                      xf_eng="gpsimd", b_eng="vector", out_queue="sync"):
    """fp8 DoubleRow variant: z ~= W8^T x8 + W8^T dx8 + dW5^T x8.

    x8 = e4m3(x), dx8 = e4m3(x - x8), W8 = e4m3(W), dW5 = e5m2(W - W8).
    Each DoubleRow pass contracts TWO k-tiles at 0.5 cyc/row, so the 12
    passes per (gate, m) cost 3072 cyc vs f32r's 4096. The scan b-term
    rebuilds f32-ish x as dequant(x8) + dequant(dx8) (0.05% error).
    """
    if chunks is None:
        chunks = [lc] * (L // lc)
    assert sum(chunks) == L and all(c % P == 0 and c <= lc for c in chunks)
    DR = mybir.MatmulPerfMode.DoubleRow

    nc = bacc.Bacc("TRN2", target_bir_lowering=False)

    x8 = nc.dram_tensor("x8", [P, KT, L], F8E4, kind="ExternalInput")
    dx8 = nc.dram_tensor("dx8", [P, KT, L], F8E4, kind="ExternalInput")
    we8 = nc.dram_tensor("we8", [P, KT, ESH], F8E4, kind="ExternalInput")
    ww8 = nc.dram_tensor("ww8", [P, KT, ESH], F8E4, kind="ExternalInput")
    dwe5 = nc.dram_tensor("dwe5", [P, KT, ESH], F8E5, kind="ExternalInput")
    dww5 = nc.dram_tensor("dww5", [P, KT, ESH], F8E5, kind="ExternalInput")
    consts = nc.dram_tensor("consts", [P, 3 * MT], F32, kind="ExternalInput")
    out = nc.dram_tensor("out", [ESH, L], F32, kind="ExternalOutput")

    with TileContext(nc) as tc:
        with (
            tc.tile_pool(name="const", bufs=1) as constp,
            tc.tile_pool(name="wsb", bufs=1) as wsb,
            tc.tile_pool(name="xt", bufs=2) as xtp,
            tc.tile_pool(name="xf", bufs=2) as xfp,
            tc.tile_pool(name="gate", bufs=3) as gatep,
            tc.tile_pool(name="bmul", bufs=4) as bmulp,
            tc.tile_pool(name="scan", bufs=2) as scanp,
            tc.tile_pool(name="ps_mm", bufs=ps_bufs, space="PSUM") as ps_mm,
        ):
            const_sb = constp.tile([P, 3 * MT], F32, tag="consts")
            bias_sb = const_sb[:, :2 * MT]
            st_sb = const_sb[:, 2 * MT:]

            x8c0 = xtp.tile([P, KT, lc], F8E4, tag="x8")
            dxc0 = xtp.tile([P, KT, lc], F8E4, tag="dx8")
            we8_sb = wsb.tile([P, KT, ESH], F8E4, tag="we8")
            ww8_sb = wsb.tile([P, KT, ESH], F8E4, tag="ww8")
            dwe5_sb = wsb.tile([P, KT, ESH], F8E5, tag="dwe5")
            dww5_sb = wsb.tile([P, KT, ESH], F8E5, tag="dww5")
            w8_sb = [we8_sb, ww8_sb]
            dw5_sb = [dwe5_sb, dww5_sb]
            nc.scalar.dma_start(we8_sb[:], we8[:])
            nc.scalar.dma_start(const_sb[:], consts[:])
            nc.sync.dma_start(x8c0[:, :4, :chunks[0]], x8[:, :4, :chunks[0]])
            nc.sync.dma_start(x8c0[:, 4:, :chunks[0]], x8[:, 4:, :chunks[0]])
            nc.scalar.dma_start(ww8_sb[:], ww8[:])
            nc.sync.dma_start(dxc0[:, :, :chunks[0]], dx8[:, :, :chunks[0]])
            nc.scalar.dma_start(dwe5_sb[:], dwe5[:])
            nc.scalar.dma_start(dww5_sb[:], dww5[:])

            prev_s = [None] * MT
            prev_lc = 0
            l0 = 0

            for c, lcc in enumerate(chunks):
                if c == 0:
                    x8c, dxc = x8c0, dxc0
                else:
                    x8c = xtp.tile([P, KT, lc], F8E4, tag="x8")
                    dxc = xtp.tile([P, KT, lc], F8E4, tag="dx8")
                    nc.sync.dma_start(x8c[:, :, :lcc], x8[:, :, l0:l0 + lcc])
                    nc.sync.dma_start(dxc[:, :, :lcc], dx8[:, :, l0:l0 + lcc])

                # f32-ish x for the scan b-term (core-own channels only)
                xf_t = xfp.tile([P, MT, lc], F32, tag="xf")
                for m in range(MT):
                    getattr(nc, xf_eng).tensor_tensor(
                        xf_t[:, m, :lcc], x8c[:, m, :lcc], dxc[:, m, :lcc],
                        op=mybir.AluOpType.add)

                for m in range(MT):
                    for gi in range(2):
                        ps = ps_mm.tile([P, lc], F32, tag="psmm")
                        for j in range(KT // 2):
                            nc.tensor.matmul(
                                ps[:, :lcc],
                                w8_sb[gi][:, 2 * j:2 * j + 2, m * P:(m + 1) * P],
                                x8c[:, 2 * j:2 * j + 2, :lcc],
                                perf_mode=DR, start=(j == 0), stop=False,
                            )
                        for j in range(KT // 2):
                            nc.tensor.matmul(
                                ps[:, :lcc],
                                w8_sb[gi][:, 2 * j:2 * j + 2, m * P:(m + 1) * P],
                                dxc[:, 2 * j:2 * j + 2, :lcc],
                                perf_mode=DR, start=False, stop=False,
                            )
                        for j in range(KT // 2):
                            nc.tensor.matmul(
                                ps[:, :lcc],
                                dw5_sb[gi][:, 2 * j:2 * j + 2, m * P:(m + 1) * P],
                                x8c[:, 2 * j:2 * j + 2, :lcc],
                                perf_mode=DR, start=False, stop=(j == KT // 2 - 1),
                            )
                        g_t = gatep.tile([P, lc], F32, tag=f"g{gi}")
                        nc.scalar.activation(
                            g_t[:, :lcc], ps[:, :lcc],
                            mybir.ActivationFunctionType.Sigmoid,
                            bias=bias_sb[:, gi * MT + m:gi * MT + m + 1],
                            scale=-1.0 if gi == 0 else 1.0,
                        )
                        if gi == 0:
                            a_t = g_t
                        else:
                            w_t = g_t

                    b_t = bmulp.tile([P, lc], F32, tag="b")
                    getattr(nc, b_eng).tensor_tensor(
                        b_t[:, :lcc], w_t[:, :lcc], xf_t[:, m, :lcc],
                        op=mybir.AluOpType.mult)

                    s_t = scanp.tile([P, lc], F32, tag=f"s{m}")
                    init = st_sb[:, m:m + 1] if c == 0 else \
                        prev_s[m][:, prev_lc - 1:prev_lc]
                    nc.vector.tensor_tensor_scan(
                        s_t[:, :lcc], a_t[:, :lcc], b_t[:, :lcc], init,
                        op0=mybir.AluOpType.mult, op1=mybir.AluOpType.add,
                    )
                    prev_s[m] = s_t

                    getattr(nc, out_queue).dma_start(
                        out[m * P:(m + 1) * P, l0:l0 + lcc], s_t[:, :lcc])

                prev_lc = lcc
                l0 += lcc

    nc.finalize()
    return nc


def _shard_inputs_fp8(x, state, erase_kernel, erase_bias, write_kernel,
                      write_bias):
    import ml_dtypes
    E4 = ml_dtypes.float8_e4m3
    E5 = ml_dtypes.float8_e5m2
    maps = []
    for core in range(8):
        b, h = divmod(core, 2)
        e0 = h * ESH
        xb = x[b].T  # [DIN, L]
        web = erase_kernel[:, e0:e0 + ESH]
        wwb = write_kernel[:, e0:e0 + ESH]
        if h == 1:
            xb = np.concatenate([xb[ESH:, :], xb[:ESH, :]], axis=0)
            web = np.concatenate([web[ESH:, :], web[:ESH, :]], axis=0)
            wwb = np.concatenate([wwb[ESH:, :], wwb[:ESH, :]], axis=0)
        x8 = xb.astype(E4)
        dx8 = (xb - x8.astype(np.float32)).astype(E4)
        we8 = web.astype(E4)
        dwe5 = (web - we8.astype(np.float32)).astype(E5)
        ww8 = wwb.astype(E4)
        dww5 = (wwb - ww8.astype(np.float32)).astype(E5)
        ben = (-erase_bias[e0:e0 + ESH]).reshape(MT, P).T
        bwp = write_bias[e0:e0 + ESH].reshape(MT, P).T
        stp = state[b, e0:e0 + ESH].reshape(MT, P).T
        maps.append({
            "x8": _pack_k(x8),
            "dx8": _pack_k(dx8),
            "we8": _pack_k(we8),
            "ww8": _pack_k(ww8),
            "dwe5": _pack_k(dwe5),
            "dww5": _pack_k(dww5),
            "consts": np.ascontiguousarray(
                np.concatenate([ben, bwp, stp], axis=1), dtype=np.float32),
        })
    return maps


_cached_nc = None

# "fp8": DoubleRow fp8 gate matmuls (2x PE matmul rate, rel err ~1.6e-3)
# "f32r": plain f32r gate matmuls (rel err ~2e-4)
VARIANT = "f32r"


def _build_kernel():
    # last chunk split in half so the scan/store tail drains while the PE
    # is still busy on the penultimate half
    chunks = [512] * 7 + [256, 256]
    if VARIANT == "fp8":
        return _build_kernel_fp8(L=L, lc=512, chunks=chunks)
    return _build_kernel_impl(L=L, lc=512, chunks=chunks)


def _pack_k(a):
    # [KT*P, N] -> [P, KT, N]
    return np.ascontiguousarray(a.reshape(KT, P, -1).transpose(1, 0, 2))


def _shard_inputs(x, state, erase_kernel, erase_bias, write_kernel, write_bias):
    if VARIANT == "fp8":
        return _shard_inputs_fp8(x, state, erase_kernel, erase_bias,
                                 write_kernel, write_bias)
    maps = []
    for core in range(8):
        b, h = divmod(core, 2)
        e0 = h * ESH
        xb = x[b].T  # [DIN, L]
        web = erase_kernel[:, e0:e0 + ESH]
        wwb = write_kernel[:, e0:e0 + ESH]
        if h == 1:
            xb = np.concatenate([xb[ESH:, :], xb[:ESH, :]], axis=0)
            web = np.concatenate([web[ESH:, :], web[:ESH, :]], axis=0)
            wwb = np.concatenate([wwb[ESH:, :], wwb[:ESH, :]], axis=0)
        ben = (-erase_bias[e0:e0 + ESH]).reshape(MT, P).T
        bwp = write_bias[e0:e0 + ESH].reshape(MT, P).T
        stp = state[b, e0:e0 + ESH].reshape(MT, P).T
        maps.append({
            "xt": _pack_k(np.asarray(xb, np.float32)),
            "we": _pack_k(np.asarray(web, np.float32)),
            "ww": _pack_k(np.asarray(wwb, np.float32)),
            "consts": np.ascontiguousarray(
                np.concatenate([ben, bwp, stp], axis=1), dtype=np.float32),
        })
    return maps


def kernel(x, state, erase_kernel, erase_bias, write_kernel, write_bias):
    global _cached_nc
    x = np.asarray(x, np.float32)
    state = np.asarray(state, np.float32)
    erase_kernel = np.asarray(erase_kernel, np.float32)
    erase_bias = np.asarray(erase_bias, np.float32)
    write_kernel = np.asarray(write_kernel, np.float32)
    write_bias = np.asarray(write_bias, np.float32)

    if _cached_nc is None:
        _cached_nc = _build_kernel()
    maps = _shard_inputs(x, state, erase_kernel, erase_bias,
                         write_kernel, write_bias)
    res = run_bass_kernel_spmd(_cached_nc, maps, core_ids=list(range(8)))
    full = np.empty((B, L, DIN), np.float32)
    for core in range(8):
        b, h = divmod(core, 2)
        full[b, :, h * ESH:(h + 1) * ESH] = res.results[core]["out"].T
    return full


# revision 15
# speedup vs baseline: 1.0929x; 1.0929x over previous
"""DeltaTokenShift Trainium2 kernel (Bass/Tile, 8 NeuronCores via axon).

Computation (per batch b):
    erase = sigmoid(x @ We + be) ; write = sigmoid(x @ Ww + bw)
    s_t = s_{t-1} * (1 - erase_t) + write_t * x_t   (scan over L, per channel)
    out[:, t, :] = s_t

Sharding: 8 cores = 4 batches x 2 halves of the 1024-channel dim. Each core
gets x[b] pre-transposed on the host to xT (the gate matmul contracts over
all 1024 input channels), its 512-column weight slices, bias/state slices,
and computes out[b][:, half] in channel-major layout [512, 4096]; the host
transposes back at gather. For upper-half cores, xT rows and weight rows are
rotated by 512 so the core's own gate channels always occupy xT k-tiles 0..3
(a consistent permutation of the contraction dim leaves the matmul result
unchanged).

Host-side packing puts the contraction k-tile index in the middle dim
([P, KT, *]) so a whole chunk of xT (all 8 k-tiles) moves with a single
dma_start, and each weight matrix with one more.

With xT supplied by the host, the PE does nothing but the gate matmuls
(f32r, 1 cyc/row): rhs tiles come straight from DMA, the b-term w*x reads
the same f32 tiles on GpSimd, the DVE scan output DMAs directly to DRAM in
[e, l] layout. No on-chip transposes, no PSUM->SBUF staging copies.

Per-core pipeline over 512-token L-chunks:
  DMA xT chunk [128, 8, 512] (sync queue, double buffered, 1 issue/chunk)
  gate matmuls in [e, l] layout accumulate in PSUM (f32r weights resident)
  ACT sigmoid straight from PSUM (erase uses scale=-1, bias=-be => 1-sigmoid)
  GpSimd b = write * xT[m] ; DVE tensor_tensor_scan(a, b) chained across
  chunks via initial=prev[:, -1:] ; DMA s tile -> out[e-block, l-chunk].
"""

import sys

sys.path.insert(0, "/opt/trn_rl_repo")

import numpy as np
import concourse.bacc as bacc
import concourse.mybir as mybir
from concourse.tile import TileContext
from concourse.bass_utils import run_bass_kernel_spmd

B, L = 4, 4096

F32 = mybir.dt.float32
F32R = mybir.dt.float32r
F8E4 = mybir.dt.float8e4
F8E5 = mybir.dt.float8e5
BF16 = mybir.dt.bfloat16

P = 128
DIN = 1024
ESH = 512
KT = DIN // P  # 8 contraction k-tiles
MT = ESH // P  # 4 output-channel groups per core


def _build_kernel_impl(L=4096, lc=512, chunks=None, ps_bufs=6, wdt=F32R,
                       xdt=F32R, b_eng="vector", out_queue="sync"):
    if chunks is None:
        chunks = [lc] * (L // lc)
    assert sum(chunks) == L and all(c % P == 0 and c <= lc for c in chunks)

    nc = bacc.Bacc("TRN2", target_bir_lowering=False)

    # Host-packed layouts: element (p, k, j) = xT[k*128 + p, j] etc.
    xt = nc.dram_tensor("xt", [P, KT, L], xdt, kind="ExternalInput")
    we = nc.dram_tensor("we", [P, KT, ESH], wdt, kind="ExternalInput")
    ww = nc.dram_tensor("ww", [P, KT, ESH], wdt, kind="ExternalInput")
    # consts[:, m] = -erase_bias group m ; [:, MT+m] = +write_bias group m ;
    # [:, 2*MT+m] = initial state group m
    consts = nc.dram_tensor("consts", [P, 3 * MT], F32, kind="ExternalInput")
    out = nc.dram_tensor("out", [ESH, L], F32, kind="ExternalOutput")

    with TileContext(nc) as tc:
        with (
            tc.tile_pool(name="const", bufs=1) as constp,
            tc.tile_pool(name="wsb", bufs=1) as wsb,
            tc.tile_pool(name="xt", bufs=2) as xtp,
            tc.tile_pool(name="gate", bufs=3) as gatep,
            tc.tile_pool(name="bmul", bufs=4) as bmulp,
            tc.tile_pool(name="scan", bufs=2) as scanp,
            tc.tile_pool(name="ps_mm", bufs=ps_bufs, space="PSUM") as ps_mm,
        ):
            const_sb = constp.tile([P, 3 * MT], F32, tag="consts")
            bias_sb = const_sb[:, :2 * MT]
            st_sb = const_sb[:, 2 * MT:]

            # Chunk-0 x k-tiles on the sync queue, weights k-tiles on the
            # scalar HWDGE queue: fine-grained interleaved arrival so the
            # first accumulation chains start as soon as their pair lands.
            xts0 = xtp.tile([P, KT, lc], xdt, tag="xt")
            we_sb = wsb.tile([P, KT, ESH], wdt, tag="w0")
            ww_sb = wsb.tile([P, KT, ESH], wdt, tag="w1")
            w_sb = [we_sb, ww_sb]
            for k in range(KT):
                nc.sync.dma_start(xts0[:, k, :chunks[0]],
                                  xt[:, k, :chunks[0]])
                for gi, wt in enumerate((we, ww)):
                    nc.scalar.dma_start(w_sb[gi][:, k, :], wt[:, k, :])
                if k == 0:
                    nc.scalar.dma_start(const_sb[:], consts[:])

            prev_s = [None] * MT
            prev_lc = 0
            l0 = 0

            for c, lcc in enumerate(chunks):
                if c == 0:
                    xts = xts0
                else:
                    xts = xtp.tile([P, KT, lc], xdt, tag="xt")
                    if c == 1:
                        # startup is DMA-bandwidth-bound: per-k granularity
                        # lets the PE chase individual k-tile arrivals
                        for k in range(KT):
                            nc.sync.dma_start(
                                xts[:, k, :lcc], xt[:, k, l0:l0 + lcc])
                    else:
                        nc.sync.dma_start(
                            xts[:, :, :lcc], xt[:, :, l0:l0 + lcc])

                for m in range(MT):
                    pe = ps_mm.tile([P, lc], F32, tag="psmm")
                    for k in range(KT):
                        nc.tensor.matmul(
                            pe[:, :lcc],
                            w_sb[0][:, k, m * P:(m + 1) * P],
                            xts[:, k, :lcc],
                            start=(k == 0), stop=(k == KT - 1),
                        )
                    a_t = gatep.tile([P, lc], F32, tag="a")
                    nc.scalar.activation(
                        a_t[:, :lcc], pe[:, :lcc],
                        mybir.ActivationFunctionType.Sigmoid,
                        bias=bias_sb[:, m:m + 1], scale=-1.0,
                    )

                    pw = ps_mm.tile([P, lc], F32, tag="psmm")
                    for k in range(KT):
                        nc.tensor.matmul(
                            pw[:, :lcc],
                            w_sb[1][:, k, m * P:(m + 1) * P],
                            xts[:, k, :lcc],
                            start=(k == 0), stop=(k == KT - 1),
                        )
                    w_t = gatep.tile([P, lc], F32, tag="w")
                    nc.scalar.activation(
                        w_t[:, :lcc], pw[:, :lcc],
                        mybir.ActivationFunctionType.Sigmoid,
                        bias=bias_sb[:, MT + m:MT + m + 1], scale=1.0,
                    )

                    b_t = bmulp.tile([P, lc], F32, tag="b")
                    getattr(nc, b_eng).tensor_tensor(
                        b_t[:, :lcc], w_t[:, :lcc],
                        xts[:, m, :lcc].bitcast(F32) if xdt is F32R
                        else xts[:, m, :lcc],
                        op=mybir.AluOpType.mult)

                    s_t = scanp.tile([P, lc], F32, tag=f"s{m}")
                    init = st_sb[:, m:m + 1] if c == 0 else \
                        prev_s[m][:, prev_lc - 1:prev_lc]
                    nc.vector.tensor_tensor_scan(
                        s_t[:, :lcc], a_t[:, :lcc], b_t[:, :lcc], init,
                        op0=mybir.AluOpType.mult, op1=mybir.AluOpType.add,
                    )
                    prev_s[m] = s_t

                    # Channel-major store straight from the scan tile; the
                    # host transposes back at gather time.
                    getattr(nc, out_queue).dma_start(
                        out[m * P:(m + 1) * P, l0:l0 + lcc], s_t[:, :lcc])

                prev_lc = lcc
                l0 += lcc

    nc.finalize()
    return nc




def _build_kernel_fp8(L=4096, lc=512, chunks=None, ps_bufs=6,
                      xf_eng="gpsimd", b_eng="vector", out_queue="sync"):
    """fp8 DoubleRow variant: z ~= W8^T x8 + W8^T dx8 + dW5^T x8.

    x8 = e4m3(x), dx8 = e4m3(x - x8), W8 = e4m3(W), dW5 = e5m2(W - W8).
    Each DoubleRow pass contracts TWO k-tiles at 0.5 cyc/row, so the 12
    passes per (gate, m) cost 3072 cyc vs f32r's 4096. The scan b-term
    rebuilds f32-ish x as dequant(x8) + dequant(dx8) (0.05% error).
    """
    if chunks is None:
        chunks = [lc] * (L // lc)
    assert sum(chunks) == L and all(c % P == 0 and c <= lc for c in chunks)
    DR = mybir.MatmulPerfMode.DoubleRow

    nc = bacc.Bacc("TRN2", target_bir_lowering=False)

    x8 = nc.dram_tensor("x8", [P, KT, L], F8E4, kind="ExternalInput")
    dx8 = nc.dram_tensor("dx8", [P, KT, L], F8E4, kind="ExternalInput")
    we8 = nc.dram_tensor("we8", [P, KT, ESH], F8E4, kind="ExternalInput")
    ww8 = nc.dram_tensor("ww8", [P, KT, ESH], F8E4, kind="ExternalInput")
    dwe5 = nc.dram_tensor("dwe5", [P, KT, ESH], F8E5, kind="ExternalInput")
    dww5 = nc.dram_tensor("dww5", [P, KT, ESH], F8E5, kind="ExternalInput")
    consts = nc.dram_tensor("consts", [P, 3 * MT], F32, kind="ExternalInput")
    out = nc.dram_tensor("out", [ESH, L], F32, kind="ExternalOutput")

    with TileContext(nc) as tc:
        with (
            tc.tile_pool(name="const", bufs=1) as constp,
            tc.tile_pool(name="wsb", bufs=1) as wsb,
            tc.tile_pool(name="xt", bufs=2) as xtp,
            tc.tile_pool(name="xf", bufs=2) as xfp,
            tc.tile_pool(name="gate", bufs=3) as gatep,
            tc.tile_pool(name="bmul", bufs=4) as bmulp,
            tc.tile_pool(name="scan", bufs=2) as scanp,
            tc.tile_pool(name="ps_mm", bufs=ps_bufs, space="PSUM") as ps_mm,
        ):
            const_sb = constp.tile([P, 3 * MT], F32, tag="consts")
            bias_sb = const_sb[:, :2 * MT]
            st_sb = const_sb[:, 2 * MT:]

            x8c0 = xtp.tile([P, KT, lc], F8E4, tag="x8")
            dxc0 = xtp.tile([P, KT, lc], F8E4, tag="dx8")
            we8_sb = wsb.tile([P, KT, ESH], F8E4, tag="we8")
            ww8_sb = wsb.tile([P, KT, ESH], F8E4, tag="ww8")
            dwe5_sb = wsb.tile([P, KT, ESH], F8E5, tag="dwe5")
            dww5_sb = wsb.tile([P, KT, ESH], F8E5, tag="dww5")
            w8_sb = [we8_sb, ww8_sb]
            dw5_sb = [dwe5_sb, dww5_sb]
            nc.scalar.dma_start(we8_sb[:], we8[:])
            nc.scalar.dma_start(const_sb[:], consts[:])
            nc.sync.dma_start(x8c0[:, :4, :chunks[0]], x8[:, :4, :chunks[0]])
            nc.sync.dma_start(x8c0[:, 4:, :chunks[0]], x8[:, 4:, :chunks[0]])
            nc.scalar.dma_start(ww8_sb[:], ww8[:])
            nc.sync.dma_start(dxc0[:, :, :chunks[0]], dx8[:, :, :chunks[0]])
            nc.scalar.dma_start(dwe5_sb[:], dwe5[:])
            nc.scalar.dma_start(dww5_sb[:], dww5[:])

            prev_s = [None] * MT
            prev_lc = 0
            l0 = 0

            for c, lcc in enumerate(chunks):
                if c == 0:
                    x8c, dxc = x8c0, dxc0
                else:
                    x8c = xtp.tile([P, KT, lc], F8E4, tag="x8")
                    dxc = xtp.tile([P, KT, lc], F8E4, tag="dx8")
                    nc.sync.dma_start(x8c[:, :, :lcc], x8[:, :, l0:l0 + lcc])
                    nc.sync.dma_start(dxc[:, :, :lcc], dx8[:, :, l0:l0 + lcc])

                # f32-ish x for the scan b-term (core-own channels only)
                xf_t = xfp.tile([P, MT, lc], F32, tag="xf")
                for m in range(MT):
                    getattr(nc, xf_eng).tensor_tensor(
                        xf_t[:, m, :lcc], x8c[:, m, :lcc], dxc[:, m, :lcc],
                        op=mybir.AluOpType.add)

                for m in range(MT):
                    for gi in range(2):
                        ps = ps_mm.tile([P, lc], F32, tag="psmm")
                        for j in range(KT // 2):
                            nc.tensor.matmul(
                                ps[:, :lcc],
                                w8_sb[gi][:, 2 * j:2 * j + 2, m * P:(m + 1) * P],
                                x8c[:, 2 * j:2 * j + 2, :lcc],
                                perf_mode=DR, start=(j == 0), stop=False,
                            )
                        for j in range(KT // 2):
                            nc.tensor.matmul(
                                ps[:, :lcc],
                                w8_sb[gi][:, 2 * j:2 * j + 2, m * P:(m + 1) * P],
                                dxc[:, 2 * j:2 * j + 2, :lcc],
                                perf_mode=DR, start=False, stop=False,
                            )
                        for j in range(KT // 2):
                            nc.tensor.matmul(
                                ps[:, :lcc],
                                dw5_sb[gi][:, 2 * j:2 * j + 2, m * P:(m + 1) * P],
                                x8c[:, 2 * j:2 * j + 2, :lcc],
                                perf_mode=DR, start=False, stop=(j == KT // 2 - 1),
                            )
                        g_t = gatep.tile([P, lc], F32, tag=f"g{gi}")
                        nc.scalar.activation(
                            g_t[:, :lcc], ps[:, :lcc],
                            mybir.ActivationFunctionType.Sigmoid,
                            bias=bias_sb[:, gi * MT + m:gi * MT + m + 1],
                            scale=-1.0 if gi == 0 else 1.0,
                        )
                        if gi == 0:
                            a_t = g_t
                        else:
                            w_t = g_t

                    b_t = bmulp.tile([P, lc], F32, tag="b")
                    getattr(nc, b_eng).tensor_tensor(
                        b_t[:, :lcc], w_t[:, :lcc], xf_t[:, m, :lcc],
                        op=mybir.AluOpType.mult)

                    s_t = scanp.tile([P, lc], F32, tag=f"s{m}")
                    init = st_sb[:, m:m + 1] if c == 0 else \
                        prev_s[m][:, prev_lc - 1:prev_lc]
                    nc.vector.tensor_tensor_scan(
                        s_t[:, :lcc], a_t[:, :lcc], b_t[:, :lcc], init,
                        op0=mybir.AluOpType.mult, op1=mybir.AluOpType.add,
                    )
                    prev_s[m] = s_t

                    getattr(nc, out_queue).dma_start(
                        out[m * P:(m + 1) * P, l0:l0 + lcc], s_t[:, :lcc])

                prev_lc = lcc
                l0 += lcc

    nc.finalize()
    return nc


def _shard_inputs_fp8(x, state, erase_kernel, erase_bias, write_kernel,
                      write_bias):
    import ml_dtypes
    E4 = ml_dtypes.float8_e4m3
    E5 = ml_dtypes.float8_e5m2
    maps = []
    for core in range(8):
        b, h = divmod(core, 2)
        e0 = h * ESH
        xb = x[b].T  # [DIN, L]
        web = erase_kernel[:, e0:e0 + ESH]
        wwb = write_kernel[:, e0:e0 + ESH]
        if h == 1:
            xb = np.concatenate([xb[ESH:, :], xb[:ESH, :]], axis=0)
            web = np.concatenate([web[ESH:, :], web[:ESH, :]], axis=0)
            wwb = np.concatenate([wwb[ESH:, :], wwb[:ESH, :]], axis=0)
        x8 = xb.astype(E4)
        dx8 = (xb - x8.astype(np.float32)).astype(E4)
        we8 = web.astype(E4)
        dwe5 = (web - we8.astype(np.float32)).astype(E5)
        ww8 = wwb.astype(E4)
        dww5 = (wwb - ww8.astype(np.float32)).astype(E5)
        ben = (-erase_bias[e0:e0 + ESH]).reshape(MT, P).T
        bwp = write_bias[e0:e0 + ESH].reshape(MT, P).T
        stp = state[b, e0:e0 + ESH].reshape(MT, P).T
        maps.append({
            "x8": _pack_k(x8),
            "dx8": _pack_k(dx8),
            "we8": _pack_k(we8),
            "ww8": _pack_k(ww8),
            "dwe5": _pack_k(dwe5),
            "dww5": _pack_k(dww5),
            "consts": np.ascontiguousarray(
                np.concatenate([ben, bwp, stp], axis=1), dtype=np.float32),
        })
    return maps


_cached_nc = None

# "fp8": DoubleRow fp8 gate matmuls (2x PE matmul rate, rel err ~1.6e-3)
# "f32r": plain f32r gate matmuls (rel err ~2e-4)
VARIANT = "bf16"


def _build_kernel():
    # last chunk split in half so the scan/store tail drains while the PE
    # is still busy on the penultimate half
    chunks = [512] * 7 + [256, 256]
    if VARIANT == "fp8":
        return _build_kernel_fp8(L=L, lc=512, chunks=chunks)
    if VARIANT == "bf16":
        return _build_kernel_impl(L=L, lc=512, chunks=chunks,
                                  wdt=BF16, xdt=BF16)
    return _build_kernel_impl(L=L, lc=512, chunks=chunks)


def _pack_k(a):
    # [KT*P, N] -> [P, KT, N]
    return np.ascontiguousarray(a.reshape(KT, P, -1).transpose(1, 0, 2))


def _shard_inputs(x, state, erase_kernel, erase_bias, write_kernel, write_bias):
    if VARIANT == "fp8":
        return _shard_inputs_fp8(x, state, erase_kernel, erase_bias,
                                 write_kernel, write_bias)
    if VARIANT == "bf16":
        import ml_dtypes
        npdt = ml_dtypes.bfloat16
    else:
        npdt = np.float32
    maps = []
    for core in range(8):
        b, h = divmod(core, 2)
        e0 = h * ESH
        xb = x[b].T  # [DIN, L]
        web = erase_kernel[:, e0:e0 + ESH]
        wwb = write_kernel[:, e0:e0 + ESH]
        if h == 1:
            xb = np.concatenate([xb[ESH:, :], xb[:ESH, :]], axis=0)
            web = np.concatenate([web[ESH:, :], web[:ESH, :]], axis=0)
            wwb = np.concatenate([wwb[ESH:, :], wwb[:ESH, :]], axis=0)
        ben = (-erase_bias[e0:e0 + ESH]).reshape(MT, P).T
        bwp = write_bias[e0:e0 + ESH].reshape(MT, P).T
        stp = state[b, e0:e0 + ESH].reshape(MT, P).T
        maps.append({
            "xt": _pack_k(np.asarray(xb, np.float32).astype(npdt)),
            "we": _pack_k(np.asarray(web, np.float32).astype(npdt)),
            "ww": _pack_k(np.asarray(wwb, np.float32).astype(npdt)),
            "consts": np.ascontiguousarray(
                np.concatenate([ben, bwp, stp], axis=1), dtype=np.float32),
        })
    return maps


def kernel(x, state, erase_kernel, erase_bias, write_kernel, write_bias):
    global _cached_nc
    x = np.asarray(x, np.float32)
    state = np.asarray(state, np.float32)
    erase_kernel = np.asarray(erase_kernel, np.float32)
    erase_bias = np.asarray(erase_bias, np.float32)
    write_kernel = np.asarray(write_kernel, np.float32)
    write_bias = np.asarray(write_bias, np.float32)

    if _cached_nc is None:
        _cached_nc = _build_kernel()
    maps = _shard_inputs(x, state, erase_kernel, erase_bias,
                         write_kernel, write_bias)
    res = run_bass_kernel_spmd(_cached_nc, maps, core_ids=list(range(8)))
    full = np.empty((B, L, DIN), np.float32)
    for core in range(8):
        b, h = divmod(core, 2)
        full[b, :, h * ESH:(h + 1) * ESH] = res.results[core]["out"].T
    return full
